# revision 27
# baseline (speedup 1.0000x reference)
"""Adaptive embedding lookup on 8 TRN2 NeuronCores.

Strategy (data-parallel over tokens, tables replicated per core):
  - input_ids is [8, 4096]; core k handles batch row k (4096 tokens).
  - Tokens are partitioned on the host by (cluster, position-band):
      cluster 0: id in [0, 20000)       -> emb0 row, copied through
      cluster 1: id in [20000, 40000)   -> emb1 row @ proj1.T
      cluster 2: id in [40000, 50000)   -> emb2 row @ proj2.T
    Each of the NBANDS position bands writes its own output DRAM tensor
    (concatenated on the host), so scatters of different bands carry no
    write-after-write deps and pipeline freely. Row BAND of each band
    tensor is a trash row for padded lanes (dropped on the host).
  - Device per (band, cluster):
      cluster 0: dma_gather emb0 rows -> SBUF -> plain indirect-DMA
                 row scatters (128 rows each) into the band tensor.
      cluster 1/2: transposed dma_gather pulls bf16 embedding rows in
                 [dim, token] layout feeding the PE matmul (lhsT)
                 directly against the bf16 projection; PSUM results go
                 to SBUF and are written out with plain indirect-DMA
                 row scatters as well (no read-modify-write traffic).
  - Padding-idx tokens (local row 1 of a table) are routed to an
    appended all-zero table row; padded lanes gather the zero row and
    scatter zeros into the band's trash row (collisions benign).
  - SPMD: one graph for all 8 cores; per-(band,cluster) lane counts are
    padded to the max across cores and bands.
"""

import os

import numpy as np

N_CORES = 8
B, S = 8, 4096
CUT0, CUT1, VOCAB = 20000, 40000, 50000
D = 1024
D1, D2 = 256, 64
PAD = 1

Z0, Z1, Z2 = 20000, 20000, 10000  # appended zero-row index per table
NBANDS = 4
BAND = S // NBANDS

LAST_EXEC_NS = None
LAST_RESULT = None


def _pack16(vals: np.ndarray, n_lanes: int, fill) -> np.ndarray:
    """Pad to n_lanes and pack int16 indices as [128, n_lanes//16]:
    index i lives at [i % 16, i // 16], replicated across the 8 GpSimd
    Q7 cores' 16-partition groups (each core reads its own group)."""
    flat = np.asarray(fill, np.int16) * np.ones(n_lanes, np.int16)
    flat[: len(vals)] = vals.astype(np.int16)
    return np.ascontiguousarray(np.tile(flat.reshape(-1, 16).T, (8, 1)))


def _pack128(vals: np.ndarray, n_lanes: int, fill) -> np.ndarray:
    """Pad to n_lanes and pack int32 as [128, n_lanes//128]: lane i at
    [i % 128, i // 128] (indirect-DMA offset layout)."""
    flat = np.asarray(fill, np.int32) * np.ones(n_lanes, np.int32)
    flat[: len(vals)] = vals.astype(np.int32)
    return np.ascontiguousarray(flat.reshape(-1, 128).T)


def _prep_core(ids_k: np.ndarray):
    out = []
    for lo, hi, zrow in ((0, CUT0, Z0), (CUT0, CUT1, Z1), (CUT1, VOCAB, Z2)):
        m = (ids_k >= lo) & (ids_k < hi)
        pos_all = np.nonzero(m)[0].astype(np.int32)
        loc_all = (ids_k[pos_all].astype(np.int64) - lo).astype(np.int32)
        loc_all[loc_all == PAD] = zrow
        bands = []
        for b in range(NBANDS):
            sel = (pos_all >= b * BAND) & (pos_all < (b + 1) * BAND)
            bands.append((loc_all[sel], pos_all[sel] - b * BAND))
        out.append(bands)
    return out


def _prepare(input_ids: np.ndarray):
    """Returns ((L0, L1, L2), in_maps)."""
    preps = [_prep_core(input_ids[k]) for k in range(N_CORES)]
    L = []
    for c in range(3):
        mx = max(len(preps[k][c][b][0]) for k in range(N_CORES) for b in range(NBANDS))
        L.append(max(1, -(-mx // 128)) * 128)

    in_maps = []
    for k in range(N_CORES):
        m = {}
        for c, zrow in ((0, Z0), (1, Z1), (2, Z2)):
            ic, qc = [], []
            for b in range(NBANDS):
                loc, pos = preps[k][c][b]
                ic.append(_pack16(loc, L[c], zrow))
                qc.append(_pack128(pos, L[c], BAND))  # pad -> trash row
            m[f"idx{c}"] = np.concatenate(ic, axis=1)
            m[f"pos{c}"] = np.concatenate(qc, axis=1)
        in_maps.append(m)
    return L, in_maps


def _build(nc, L0: int, L1: int, L2: int):
    from concourse import library_config, mybir, tile
    from concourse.bass import IndirectOffsetOnAxis

    f32 = mybir.dt.float32
    bf16 = mybir.dt.bfloat16
    i16 = mybir.dt.int16
    i32 = mybir.dt.int32

    n0, n1, n2 = L0 // 128, L1 // 128, L2 // 128
    W0, W1, W2 = L0 // 16, L1 // 16, L2 // 16

    emb0p = nc.dram_tensor("emb0p", [Z0 + 1, D], bf16, kind="ExternalInput")
    emb1b = nc.dram_tensor("emb1b", [Z1 + 1, D1], bf16, kind="ExternalInput")
    emb2b = nc.dram_tensor("emb2b", [Z2 + 1, 128], bf16, kind="ExternalInput")
    p1t = nc.dram_tensor("p1t", [D1, D], bf16, kind="ExternalInput")
    p2t = nc.dram_tensor("p2t", [128, D], bf16, kind="ExternalInput")
    idx0 = nc.dram_tensor("idx0", [128, NBANDS * W0], i16, kind="ExternalInput")
    pos0 = nc.dram_tensor("pos0", [128, NBANDS * n0], i32, kind="ExternalInput")
    idx1 = nc.dram_tensor("idx1", [128, NBANDS * W1], i16, kind="ExternalInput")
    pos1 = nc.dram_tensor("pos1", [128, NBANDS * n1], i32, kind="ExternalInput")
    idx2 = nc.dram_tensor("idx2", [128, NBANDS * W2], i16, kind="ExternalInput")
    pos2 = nc.dram_tensor("pos2", [128, NBANDS * n2], i32, kind="ExternalInput")
    outs = [
        nc.dram_tensor(f"out{b}", [BAND + 1, D], f32, kind="ExternalOutput")
        for b in range(NBANDS)
    ]

    nc.gpsimd.load_library(library_config.mlp)

    with tile.TileContext(nc) as tc:
        with (
            tc.tile_pool(name="const", bufs=1) as cpool,
            tc.tile_pool(name="gA", bufs=2) as gapool,
            tc.tile_pool(name="gB", bufs=2) as gbpool,
            tc.tile_pool(name="o", bufs=3) as opool,
            tc.tile_pool(name="po", bufs=4, space="PSUM") as popool,
        ):
            sb = {}
            for nm, t in (
                ("idx0", idx0),
                ("pos0", pos0),
                ("idx1", idx1),
                ("pos1", pos1),
                ("idx2", idx2),
                ("pos2", pos2),
            ):
                s = cpool.tile(list(t.shape), t.dtype, name=f"{nm}_sb")
                nc.sync.dma_start(out=s[:], in_=t[:])
                sb[nm] = s

            p1b = cpool.tile([128, 2, D], bf16)
            nc.sync.dma_start(out=p1b[:, 0, :], in_=p1t[0:128, :])
            nc.sync.dma_start(out=p1b[:, 1, :], in_=p1t[128:256, :])
            p2b = cpool.tile([128, 1, D], bf16)
            nc.sync.dma_start(out=p2b[:, 0, :], in_=p2t[:])

            for b in range(NBANDS):
                band = outs[b]

                # ---- cluster 0: gather f32 rows, plain indirect scatters ----
                gA = gapool.tile([128, n0, D], bf16, tag="gA", name=f"gA_{b}")
                nc.gpsimd.dma_gather(
                    gA[:],
                    emb0p[:],
                    sb["idx0"][:, b * W0 : (b + 1) * W0],
                    L0,
                    L0,
                    D,
                )
                for j in range(n0):
                    nc.gpsimd.indirect_dma_start(
                        out=band[:],
                        out_offset=IndirectOffsetOnAxis(
                            ap=sb["pos0"][:, b * n0 + j : b * n0 + j + 1], axis=0
                        ),
                        in_=gA[:, j, :],
                        in_offset=None,
                    )

                # ---- clusters 1/2: transposed gather -> matmul -> scatter ----
                for c, (n_t, w, kch, table, pb, elem, lanes) in enumerate(
                    (
                        (n1, W1, 2, emb1b, p1b, D1, L1),
                        (n2, W2, 1, emb2b, p2b, 128, L2),
                    ),
                    start=1,
                ):
                    gB = gbpool.tile(
                        [128, kch, lanes], bf16, tag=f"gB{c}", name=f"gB{c}_{b}"
                    )
                    nc.gpsimd.dma_gather(
                        gB[:],
                        table[:],
                        sb[f"idx{c}"][:, b * w : (b + 1) * w],
                        lanes,
                        lanes,
                        elem,
                        transpose=True,
                    )
                    oc = opool.tile(
                        [128, n_t, D], bf16, tag=f"oc{c}", name=f"oc{c}_{b}"
                    )
                    for t in range(n_t):
                        for nn in range(2):
                            om = popool.tile([128, 512], f32, tag="om", name="om")
                            for kc in range(kch):
                                nc.tensor.matmul(
                                    out=om[:],
                                    lhsT=gB[:, kc, t * 128 : (t + 1) * 128],
                                    rhs=pb[:, kc, nn * 512 : (nn + 1) * 512],
                                    start=(kc == 0),
                                    stop=(kc == kch - 1),
                                )
                            dst = oc[:, t, nn * 512 : (nn + 1) * 512]
                            if nn == 0:
                                nc.scalar.copy(out=dst, in_=om[:])
                            else:
                                nc.vector.tensor_copy(out=dst, in_=om[:])
                    for t in range(n_t):
                        nc.gpsimd.indirect_dma_start(
                            out=band[:],
                            out_offset=IndirectOffsetOnAxis(
                                ap=sb[f"pos{c}"][
                                    :, b * n_t + t : b * n_t + t + 1
                                ],
                                axis=0,
                            ),
                            in_=oc[:, t, :],
                            in_offset=None,
                        )

    return outs


def kernel(input_ids, emb0, emb1, emb2, proj1, proj2):
    global LAST_EXEC_NS, LAST_RESULT
    import ml_dtypes
    from concourse import bacc
    from concourse.bass_utils import run_bass_kernel_spmd

    bf = ml_dtypes.bfloat16
    input_ids = np.asarray(input_ids)
    assert input_ids.shape == (B, S), input_ids.shape

    emb0p = np.concatenate([emb0, np.zeros((1, D), np.float32)], axis=0).astype(bf)
    emb1b = np.concatenate([emb1, np.zeros((1, D1), np.float32)], axis=0).astype(bf)
    emb2b = np.zeros((Z2 + 1, 128), dtype=bf)
    emb2b[:Z2, :D2] = emb2.astype(bf)
    p1t = np.ascontiguousarray(proj1.T).astype(bf)
    p2t = np.zeros((128, D), dtype=bf)
    p2t[:D2] = np.ascontiguousarray(proj2.T).astype(bf)

    (L0, L1, L2), in_maps = _prepare(input_ids)
    tables = {
        "emb0p": emb0p,
        "emb1b": emb1b,
        "emb2b": emb2b,
        "p1t": p1t,
        "p2t": p2t,
    }
    for m in in_maps:
        m.update(tables)

    nc = bacc.Bacc("TRN2", target_bir_lowering=False, debug=False, num_devices=N_CORES)
    _build(nc, L0, L1, L2)
    nc.compile()

    trace = bool(os.environ.get("EMB_KERNEL_TRACE"))
    res = run_bass_kernel_spmd(nc, in_maps, list(range(N_CORES)), trace=trace)
    LAST_RESULT = res
    LAST_EXEC_NS = res.exec_time_ns

    out = np.stack(
        [
            np.concatenate(
                [
                    np.asarray(res.results[k][f"out{b}"]).reshape(BAND + 1, D)[:BAND]
                    for b in range(NBANDS)
                ],
                axis=0,
            )
            for k in range(N_CORES)
        ],
        axis=0,
    )
    return out


# revision 28
# speedup vs baseline: 1.0692x; 1.0692x over previous
"""Adaptive embedding lookup on 8 TRN2 NeuronCores.

Strategy (data-parallel over tokens, tables replicated per core):
  - input_ids is [8, 4096]; core k handles batch row k (4096 tokens).
  - Tokens are partitioned on the host by (cluster, position-band):
      cluster 0: id in [0, 20000)       -> emb0 row, copied through
      cluster 1: id in [20000, 40000)   -> emb1 row @ proj1.T
      cluster 2: id in [40000, 50000)   -> emb2 row @ proj2.T
    Each of the NBANDS position bands writes its own output DRAM tensor
    (concatenated on the host), so scatters of different bands carry no
    write-after-write deps and pipeline freely. Row BAND of each band
    tensor is a trash row for padded lanes (dropped on the host).
  - Device per (band, cluster):
      cluster 0: dma_gather emb0 rows -> SBUF -> plain indirect-DMA
                 row scatters (128 rows each) into the band tensor.
      cluster 1/2: transposed dma_gather pulls bf16 embedding rows in
                 [dim, token] layout feeding the PE matmul (lhsT)
                 directly against the bf16 projection; PSUM results go
                 to SBUF and are written out with plain indirect-DMA
                 row scatters as well (no read-modify-write traffic).
  - Padding-idx tokens (local row 1 of a table) are routed to an
    appended all-zero table row; padded lanes gather the zero row and
    scatter zeros into the band's trash row (collisions benign).
  - SPMD: one graph for all 8 cores; per-(band,cluster) lane counts are
    padded to the max across cores and bands.
"""

import os

import numpy as np

N_CORES = 8
B, S = 8, 4096
CUT0, CUT1, VOCAB = 20000, 40000, 50000
D = 1024
D1, D2 = 256, 64
PAD = 1

Z0, Z1, Z2 = 20000, 20000, 10000  # appended zero-row index per table
NBANDS = 4
BAND = S // NBANDS

LAST_EXEC_NS = None
LAST_RESULT = None


def _pack16(vals: np.ndarray, n_lanes: int, fill) -> np.ndarray:
    """Pad to n_lanes and pack int16 indices as [128, n_lanes//16]:
    index i lives at [i % 16, i // 16], replicated across the 8 GpSimd
    Q7 cores' 16-partition groups (each core reads its own group)."""
    flat = np.asarray(fill, np.int16) * np.ones(n_lanes, np.int16)
    flat[: len(vals)] = vals.astype(np.int16)
    return np.ascontiguousarray(np.tile(flat.reshape(-1, 16).T, (8, 1)))


def _pack128(vals: np.ndarray, n_lanes: int, fill) -> np.ndarray:
    """Pad to n_lanes and pack int32 as [128, n_lanes//128]: lane i at
    [i % 128, i // 128] (indirect-DMA offset layout)."""
    flat = np.asarray(fill, np.int32) * np.ones(n_lanes, np.int32)
    flat[: len(vals)] = vals.astype(np.int32)
    return np.ascontiguousarray(flat.reshape(-1, 128).T)


def _prep_core(ids_k: np.ndarray):
    out = []
    for lo, hi, zrow in ((0, CUT0, Z0), (CUT0, CUT1, Z1), (CUT1, VOCAB, Z2)):
        m = (ids_k >= lo) & (ids_k < hi)
        pos_all = np.nonzero(m)[0].astype(np.int32)
        loc_all = (ids_k[pos_all].astype(np.int64) - lo).astype(np.int32)
        loc_all[loc_all == PAD] = zrow
        bands = []
        for b in range(NBANDS):
            sel = (pos_all >= b * BAND) & (pos_all < (b + 1) * BAND)
            bands.append((loc_all[sel], pos_all[sel] - b * BAND))
        out.append(bands)
    return out


def _prepare(input_ids: np.ndarray):
    """Returns ((L0, L1, L2), in_maps)."""
    preps = [_prep_core(input_ids[k]) for k in range(N_CORES)]
    L = []
    for c in range(3):
        mx = max(len(preps[k][c][b][0]) for k in range(N_CORES) for b in range(NBANDS))
        L.append(max(1, -(-mx // 128)) * 128)

    in_maps = []
    for k in range(N_CORES):
        m = {}
        for c, zrow in ((0, Z0), (1, Z1), (2, Z2)):
            ic, qc = [], []
            for b in range(NBANDS):
                loc, pos = preps[k][c][b]
                ic.append(_pack16(loc, L[c], zrow))
                qc.append(_pack128(pos, L[c], BAND))  # pad -> trash row
            m[f"idx{c}"] = np.concatenate(ic, axis=1)
            m[f"pos{c}"] = np.concatenate(qc, axis=1)
        in_maps.append(m)
    return L, in_maps


def _build(nc, L0: int, L1: int, L2: int):
    from concourse import library_config, mybir, tile
    from concourse.bass import IndirectOffsetOnAxis

    f32 = mybir.dt.float32
    bf16 = mybir.dt.bfloat16
    i16 = mybir.dt.int16
    i32 = mybir.dt.int32

    n0, n1, n2 = L0 // 128, L1 // 128, L2 // 128
    W0, W1, W2 = L0 // 16, L1 // 16, L2 // 16

    emb0p = nc.dram_tensor("emb0p", [Z0 + 1, D], bf16, kind="ExternalInput")
    emb1b = nc.dram_tensor("emb1b", [Z1 + 1, D1], bf16, kind="ExternalInput")
    emb2b = nc.dram_tensor("emb2b", [Z2 + 1, 128], bf16, kind="ExternalInput")
    p1t = nc.dram_tensor("p1t", [D1, D], bf16, kind="ExternalInput")
    p2t = nc.dram_tensor("p2t", [128, D], bf16, kind="ExternalInput")
    idx0 = nc.dram_tensor("idx0", [128, NBANDS * W0], i16, kind="ExternalInput")
    pos0 = nc.dram_tensor("pos0", [128, NBANDS * n0], i32, kind="ExternalInput")
    idx1 = nc.dram_tensor("idx1", [128, NBANDS * W1], i16, kind="ExternalInput")
    pos1 = nc.dram_tensor("pos1", [128, NBANDS * n1], i32, kind="ExternalInput")
    idx2 = nc.dram_tensor("idx2", [128, NBANDS * W2], i16, kind="ExternalInput")
    pos2 = nc.dram_tensor("pos2", [128, NBANDS * n2], i32, kind="ExternalInput")
    outs = [
        nc.dram_tensor(f"out{b}", [BAND + 1, D], f32, kind="ExternalOutput")
        for b in range(NBANDS)
    ]

    nc.gpsimd.load_library(library_config.mlp)

    with tile.TileContext(nc) as tc:
        with (
            tc.tile_pool(name="const", bufs=1) as cpool,
            tc.tile_pool(name="gA", bufs=2) as gapool,
            tc.tile_pool(name="gB", bufs=2) as gbpool,
            tc.tile_pool(name="o", bufs=3) as opool,
            tc.tile_pool(name="po", bufs=4, space="PSUM") as popool,
        ):
            sb = {}
            for nm, t in (
                ("idx0", idx0),
                ("pos0", pos0),
                ("idx1", idx1),
                ("pos1", pos1),
                ("idx2", idx2),
                ("pos2", pos2),
            ):
                s = cpool.tile(list(t.shape), t.dtype, name=f"{nm}_sb")
                nc.sync.dma_start(out=s[:], in_=t[:])
                sb[nm] = s

            p1b = cpool.tile([128, 2, D], bf16)
            nc.sync.dma_start(out=p1b[:, 0, :], in_=p1t[0:128, :])
            nc.sync.dma_start(out=p1b[:, 1, :], in_=p1t[128:256, :])
            p2b = cpool.tile([128, 1, D], bf16)
            nc.sync.dma_start(out=p2b[:, 0, :], in_=p2t[:])

            for b in range(NBANDS):
                band = outs[b]

                # ---- cluster 0: gather f32 rows, plain indirect scatters ----
                gA = gapool.tile([128, n0, D], bf16, tag="gA", name=f"gA_{b}")
                nc.gpsimd.dma_gather(
                    gA[:],
                    emb0p[:],
                    sb["idx0"][:, b * W0 : (b + 1) * W0],
                    L0,
                    L0,
                    D,
                )
                for j in range(n0):
                    nc.gpsimd.indirect_dma_start(
                        out=band[:],
                        out_offset=IndirectOffsetOnAxis(
                            ap=sb["pos0"][:, b * n0 + j : b * n0 + j + 1], axis=0
                        ),
                        in_=gA[:, j, :],
                        in_offset=None,
                    )

                # ---- clusters 1/2: transposed gather -> matmul -> scatter ----
                for c, (n_t, w, kch, table, pb, elem, lanes) in enumerate(
                    (
                        (n1, W1, 2, emb1b, p1b, D1, L1),
                        (n2, W2, 1, emb2b, p2b, 128, L2),
                    ),
                    start=1,
                ):
                    gB = gbpool.tile(
                        [128, kch, lanes], bf16, tag=f"gB{c}", name=f"gB{c}_{b}"
                    )
                    nc.gpsimd.dma_gather(
                        gB[:],
                        table[:],
                        sb[f"idx{c}"][:, b * w : (b + 1) * w],
                        lanes,
                        lanes,
                        elem,
                        transpose=True,
                    )
                    oc = opool.tile(
                        [128, n_t, D], f32, tag=f"oc{c}", name=f"oc{c}_{b}"
                    )
                    for t in range(n_t):
                        for nn in range(2):
                            om = popool.tile([128, 512], f32, tag="om", name="om")
                            for kc in range(kch):
                                nc.tensor.matmul(
                                    out=om[:],
                                    lhsT=gB[:, kc, t * 128 : (t + 1) * 128],
                                    rhs=pb[:, kc, nn * 512 : (nn + 1) * 512],
                                    start=(kc == 0),
                                    stop=(kc == kch - 1),
                                )
                            dst = oc[:, t, nn * 512 : (nn + 1) * 512]
                            if nn == 0:
                                nc.scalar.copy(out=dst, in_=om[:])
                            else:
                                nc.vector.tensor_copy(out=dst, in_=om[:])
                    for t in range(n_t):
                        nc.gpsimd.indirect_dma_start(
                            out=band[:],
                            out_offset=IndirectOffsetOnAxis(
                                ap=sb[f"pos{c}"][
                                    :, b * n_t + t : b * n_t + t + 1
                                ],
                                axis=0,
                            ),
                            in_=oc[:, t, :],
                            in_offset=None,
                        )

    return outs


def kernel(input_ids, emb0, emb1, emb2, proj1, proj2):
    global LAST_EXEC_NS, LAST_RESULT
    import ml_dtypes
    from concourse import bacc
    from concourse.bass_utils import run_bass_kernel_spmd

    bf = ml_dtypes.bfloat16
    input_ids = np.asarray(input_ids)
    assert input_ids.shape == (B, S), input_ids.shape

    emb0p = np.concatenate([emb0, np.zeros((1, D), np.float32)], axis=0).astype(bf)
    emb1b = np.concatenate([emb1, np.zeros((1, D1), np.float32)], axis=0).astype(bf)
    emb2b = np.zeros((Z2 + 1, 128), dtype=bf)
    emb2b[:Z2, :D2] = emb2.astype(bf)
    p1t = np.ascontiguousarray(proj1.T).astype(bf)
    p2t = np.zeros((128, D), dtype=bf)
    p2t[:D2] = np.ascontiguousarray(proj2.T).astype(bf)

    (L0, L1, L2), in_maps = _prepare(input_ids)
    tables = {
        "emb0p": emb0p,
        "emb1b": emb1b,
        "emb2b": emb2b,
        "p1t": p1t,
        "p2t": p2t,
    }
    for m in in_maps:
        m.update(tables)

    nc = bacc.Bacc("TRN2", target_bir_lowering=False, debug=False, num_devices=N_CORES)
    _build(nc, L0, L1, L2)
    nc.compile()

    trace = bool(os.environ.get("EMB_KERNEL_TRACE"))
    res = run_bass_kernel_spmd(nc, in_maps, list(range(N_CORES)), trace=trace)
    LAST_RESULT = res
    LAST_EXEC_NS = res.exec_time_ns

    out = np.stack(
        [
            np.concatenate(
                [
                    np.asarray(res.results[k][f"out{b}"]).reshape(BAND + 1, D)[:BAND]
                    for b in range(NBANDS)
                ],
                axis=0,
            )
            for k in range(N_CORES)
        ],
        axis=0,
    )
    return out


# revision 31
# speedup vs baseline: 1.1736x; 1.0977x over previous
"""Adaptive embedding lookup on 8 TRN2 NeuronCores.

Strategy (data-parallel over tokens, tables replicated per core):
  - input_ids is [8, 4096]; core k handles batch row k (4096 tokens).
  - Tokens are partitioned on the host by (cluster, position-band):
      cluster 0: id in [0, 20000)       -> emb0 row, copied through
      cluster 1: id in [20000, 40000)   -> emb1 row @ proj1.T
      cluster 2: id in [40000, 50000)   -> emb2 row @ proj2.T
    Each of the NBANDS position bands writes its own output DRAM tensor
    (concatenated on the host), so scatters of different bands carry no
    write-after-write deps and pipeline freely. Row BAND of each band
    tensor is a trash row for padded lanes (dropped on the host).
  - Device per (band, cluster):
      cluster 0: dma_gather emb0 rows -> SBUF -> plain indirect-DMA
                 row scatters (128 rows each) into the band tensor.
      cluster 1/2: transposed dma_gather pulls bf16 embedding rows in
                 [dim, token] layout feeding the PE matmul (lhsT)
                 directly against the bf16 projection; PSUM results go
                 to SBUF and are written out with plain indirect-DMA
                 row scatters as well (no read-modify-write traffic).
  - Padding-idx tokens (local row 1 of a table) are routed to an
    appended all-zero table row; padded lanes gather the zero row and
    scatter zeros into the band's trash row (collisions benign).
  - SPMD: one graph for all 8 cores; per-(band,cluster) lane counts are
    padded to the max across cores and bands.
"""

import os

import numpy as np

N_CORES = 8
B, S = 8, 4096
CUT0, CUT1, VOCAB = 20000, 40000, 50000
D = 1024
D1, D2 = 256, 64
PAD = 1

Z0, Z1, Z2 = 20000, 20000, 10000  # appended zero-row index per table
NBANDS = 4
BOUNDS = [0, 1216, 2432, 3648, S]  # short last band -> short tail chain
BSZ = [BOUNDS[i + 1] - BOUNDS[i] for i in range(NBANDS)]
BAND = S // NBANDS  # legacy trash-row fill for _pack helpers

LAST_EXEC_NS = None
LAST_RESULT = None


def _pack16(vals: np.ndarray, n_lanes: int, fill) -> np.ndarray:
    """Pad to n_lanes and pack int16 indices as [128, n_lanes//16]:
    index i lives at [i % 16, i // 16], replicated across the 8 GpSimd
    Q7 cores' 16-partition groups (each core reads its own group)."""
    flat = np.asarray(fill, np.int16) * np.ones(n_lanes, np.int16)
    flat[: len(vals)] = vals.astype(np.int16)
    return np.ascontiguousarray(np.tile(flat.reshape(-1, 16).T, (8, 1)))


def _pack128(vals: np.ndarray, n_lanes: int, fill) -> np.ndarray:
    """Pad to n_lanes and pack int32 as [128, n_lanes//128]: lane i at
    [i % 128, i // 128] (indirect-DMA offset layout)."""
    flat = np.asarray(fill, np.int32) * np.ones(n_lanes, np.int32)
    flat[: len(vals)] = vals.astype(np.int32)
    return np.ascontiguousarray(flat.reshape(-1, 128).T)


def _prep_core(ids_k: np.ndarray):
    out = []
    for lo, hi, zrow in ((0, CUT0, Z0), (CUT0, CUT1, Z1), (CUT1, VOCAB, Z2)):
        m = (ids_k >= lo) & (ids_k < hi)
        pos_all = np.nonzero(m)[0].astype(np.int32)
        loc_all = (ids_k[pos_all].astype(np.int64) - lo).astype(np.int32)
        loc_all[loc_all == PAD] = zrow
        bands = []
        for b in range(NBANDS):
            sel = (pos_all >= BOUNDS[b]) & (pos_all < BOUNDS[b + 1])
            bands.append((loc_all[sel], pos_all[sel] - BOUNDS[b]))
        out.append(bands)
    return out


def _prepare(input_ids: np.ndarray):
    """Returns ((L0, L1, L2), in_maps)."""
    preps = [_prep_core(input_ids[k]) for k in range(N_CORES)]
    L = []
    for c in range(3):
        L.append(
            [
                max(
                    1,
                    -(-max(len(preps[k][c][b][0]) for k in range(N_CORES)) // 128),
                )
                * 128
                for b in range(NBANDS)
            ]
        )

    in_maps = []
    for k in range(N_CORES):
        m = {}
        for c, zrow in ((0, Z0), (1, Z1), (2, Z2)):
            ic, qc = [], []
            for b in range(NBANDS):
                loc, pos = preps[k][c][b]
                ic.append(_pack16(loc, L[c][b], zrow))
                qc.append(_pack128(pos, L[c][b], BSZ[b]))  # pad -> trash row
            m[f"idx{c}"] = np.concatenate(ic, axis=1)
            m[f"pos{c}"] = np.concatenate(qc, axis=1)
        in_maps.append(m)
    return L, in_maps


def _build(nc, L0: int, L1: int, L2: int):
    from concourse import library_config, mybir, tile
    from concourse.bass import IndirectOffsetOnAxis

    f32 = mybir.dt.float32
    bf16 = mybir.dt.bfloat16
    i16 = mybir.dt.int16
    i32 = mybir.dt.int32

    Ls = {0: L0, 1: L1, 2: L2}  # per-band lane counts per cluster
    # prefix sums for idx (cols of 16 lanes) and pos (cols of 128 lanes)
    ioff = {c: [sum(Ls[c][:b]) // 16 for b in range(NBANDS + 1)] for c in Ls}
    poff = {c: [sum(Ls[c][:b]) // 128 for b in range(NBANDS + 1)] for c in Ls}

    emb0p = nc.dram_tensor("emb0p", [Z0 + 1, D], bf16, kind="ExternalInput")
    emb1b = nc.dram_tensor("emb1b", [Z1 + 1, D1], bf16, kind="ExternalInput")
    emb2b = nc.dram_tensor("emb2b", [Z2 + 1, 128], bf16, kind="ExternalInput")
    p1t = nc.dram_tensor("p1t", [D1, D], bf16, kind="ExternalInput")
    p2t = nc.dram_tensor("p2t", [128, D], bf16, kind="ExternalInput")
    idx0 = nc.dram_tensor("idx0", [128, ioff[0][-1]], i16, kind="ExternalInput")
    pos0 = nc.dram_tensor("pos0", [128, poff[0][-1]], i32, kind="ExternalInput")
    idx1 = nc.dram_tensor("idx1", [128, ioff[1][-1]], i16, kind="ExternalInput")
    pos1 = nc.dram_tensor("pos1", [128, poff[1][-1]], i32, kind="ExternalInput")
    idx2 = nc.dram_tensor("idx2", [128, ioff[2][-1]], i16, kind="ExternalInput")
    pos2 = nc.dram_tensor("pos2", [128, poff[2][-1]], i32, kind="ExternalInput")
    outs = [
        nc.dram_tensor(f"out{b}", [BSZ[b] + 1, D], f32, kind="ExternalOutput")
        for b in range(NBANDS)
    ]

    nc.gpsimd.load_library(library_config.mlp)

    with tile.TileContext(nc) as tc:
        with (
            tc.tile_pool(name="const", bufs=1) as cpool,
            tc.tile_pool(name="gA", bufs=2) as gapool,
            tc.tile_pool(name="gB", bufs=2) as gbpool,
            tc.tile_pool(name="o", bufs=3) as opool,
            tc.tile_pool(name="po", bufs=4, space="PSUM") as popool,
        ):
            sb = {}
            for nm, t in (
                ("idx0", idx0),
                ("pos0", pos0),
                ("idx1", idx1),
                ("pos1", pos1),
                ("idx2", idx2),
                ("pos2", pos2),
            ):
                s = cpool.tile(list(t.shape), t.dtype, name=f"{nm}_sb")
                nc.sync.dma_start(out=s[:], in_=t[:])
                sb[nm] = s

            p1b = cpool.tile([128, 2, D], bf16)
            nc.sync.dma_start(out=p1b[:, 0, :], in_=p1t[0:128, :])
            nc.sync.dma_start(out=p1b[:, 1, :], in_=p1t[128:256, :])
            p2b = cpool.tile([128, 1, D], bf16)
            nc.sync.dma_start(out=p2b[:, 0, :], in_=p2t[:])

            for b in range(NBANDS):
                band = outs[b]
                n0b = Ls[0][b] // 128

                # ---- cluster 0: gather bf16 rows, plain indirect scatters ----
                gA = gapool.tile([128, n0b, D], bf16, tag="gA", name=f"gA_{b}")
                nc.gpsimd.dma_gather(
                    gA[:],
                    emb0p[:],
                    sb["idx0"][:, ioff[0][b] : ioff[0][b + 1]],
                    Ls[0][b],
                    Ls[0][b],
                    D,
                )
                for j in range(n0b):
                    nc.gpsimd.indirect_dma_start(
                        out=band[:],
                        out_offset=IndirectOffsetOnAxis(
                            ap=sb["pos0"][:, poff[0][b] + j : poff[0][b] + j + 1],
                            axis=0,
                        ),
                        in_=gA[:, j, :],
                        in_offset=None,
                    )

                # ---- clusters 1/2: transposed gather -> matmul -> scatter ----
                for c, (kch, table, pb, elem) in enumerate(
                    (
                        (2, emb1b, p1b, D1),
                        (1, emb2b, p2b, 128),
                    ),
                    start=1,
                ):
                    lanes = Ls[c][b]
                    n_t = lanes // 128
                    gB = gbpool.tile(
                        [128, kch, lanes], bf16, tag=f"gB{c}", name=f"gB{c}_{b}"
                    )
                    nc.gpsimd.dma_gather(
                        gB[:],
                        table[:],
                        sb[f"idx{c}"][:, ioff[c][b] : ioff[c][b + 1]],
                        lanes,
                        lanes,
                        elem,
                        transpose=True,
                    )
                    oc = opool.tile(
                        [128, n_t, D], f32, tag=f"oc{c}", name=f"oc{c}_{b}"
                    )
                    for t in range(n_t):
                        for nn in range(2):
                            om = popool.tile([128, 512], f32, tag="om", name="om")
                            for kc in range(kch):
                                nc.tensor.matmul(
                                    out=om[:],
                                    lhsT=gB[:, kc, t * 128 : (t + 1) * 128],
                                    rhs=pb[:, kc, nn * 512 : (nn + 1) * 512],
                                    start=(kc == 0),
                                    stop=(kc == kch - 1),
                                )
                            dst = oc[:, t, nn * 512 : (nn + 1) * 512]
                            if nn == 0:
                                nc.scalar.copy(out=dst, in_=om[:])
                            else:
                                nc.vector.tensor_copy(out=dst, in_=om[:])
                    for t in range(n_t):
                        nc.gpsimd.indirect_dma_start(
                            out=band[:],
                            out_offset=IndirectOffsetOnAxis(
                                ap=sb[f"pos{c}"][
                                    :, poff[c][b] + t : poff[c][b] + t + 1
                                ],
                                axis=0,
                            ),
                            in_=oc[:, t, :],
                            in_offset=None,
                        )

    return outs


def kernel(input_ids, emb0, emb1, emb2, proj1, proj2):
    global LAST_EXEC_NS, LAST_RESULT
    import ml_dtypes
    from concourse import bacc
    from concourse.bass_utils import run_bass_kernel_spmd

    bf = ml_dtypes.bfloat16
    input_ids = np.asarray(input_ids)
    assert input_ids.shape == (B, S), input_ids.shape

    emb0p = np.concatenate([emb0, np.zeros((1, D), np.float32)], axis=0).astype(bf)
    emb1b = np.concatenate([emb1, np.zeros((1, D1), np.float32)], axis=0).astype(bf)
    emb2b = np.zeros((Z2 + 1, 128), dtype=bf)
    emb2b[:Z2, :D2] = emb2.astype(bf)
    p1t = np.ascontiguousarray(proj1.T).astype(bf)
    p2t = np.zeros((128, D), dtype=bf)
    p2t[:D2] = np.ascontiguousarray(proj2.T).astype(bf)

    (L0, L1, L2), in_maps = _prepare(input_ids)
    tables = {
        "emb0p": emb0p,
        "emb1b": emb1b,
        "emb2b": emb2b,
        "p1t": p1t,
        "p2t": p2t,
    }
    for m in in_maps:
        m.update(tables)

    nc = bacc.Bacc("TRN2", target_bir_lowering=False, debug=False, num_devices=N_CORES)
    _build(nc, L0, L1, L2)
    nc.compile()

    trace = bool(os.environ.get("EMB_KERNEL_TRACE"))
    res = run_bass_kernel_spmd(nc, in_maps, list(range(N_CORES)), trace=trace)
    LAST_RESULT = res
    LAST_EXEC_NS = res.exec_time_ns

    out = np.stack(
        [
            np.concatenate(
                [
                    np.asarray(res.results[k][f"out{b}"]).reshape(BSZ[b] + 1, D)[: BSZ[b]]
                    for b in range(NBANDS)
                ],
                axis=0,
            )
            for k in range(N_CORES)
        ],
        axis=0,
    )
    return out


# revision 34
# speedup vs baseline: 1.2630x; 1.0762x over previous
"""Adaptive embedding lookup on 8 TRN2 NeuronCores.

Strategy (data-parallel over tokens, tables replicated per core):
  - input_ids is [8, 4096]; core k handles batch row k (4096 tokens).
  - Tokens are partitioned on the host by (cluster, position-band):
      cluster 0: id in [0, 20000)       -> emb0 row, copied through
      cluster 1: id in [20000, 40000)   -> emb1 row @ proj1.T
      cluster 2: id in [40000, 50000)   -> emb2 row @ proj2.T
    Each of the NBANDS position bands writes its own output DRAM tensor
    (concatenated on the host), so scatters of different bands carry no
    write-after-write deps and pipeline freely. Row BAND of each band
    tensor is a trash row for padded lanes (dropped on the host).
  - Device per (band, cluster):
      cluster 0: dma_gather emb0 rows -> SBUF -> plain indirect-DMA
                 row scatters (128 rows each) into the band tensor.
      cluster 1/2: transposed dma_gather pulls bf16 embedding rows in
                 [dim, token] layout feeding the PE matmul (lhsT)
                 directly against the bf16 projection; PSUM results go
                 to SBUF and are written out with plain indirect-DMA
                 row scatters as well (no read-modify-write traffic).
  - Padding-idx tokens (local row 1 of a table) are routed to an
    appended all-zero table row; padded lanes gather the zero row and
    scatter zeros into the band's trash row (collisions benign).
  - SPMD: one graph for all 8 cores; per-(band,cluster) lane counts are
    padded to the max across cores and bands.
"""

import os

import numpy as np

N_CORES = 8
B, S = 8, 4096
CUT0, CUT1, VOCAB = 20000, 40000, 50000
D = 1024
D1, D2 = 256, 64
PAD = 1

Z0, Z1, Z2 = 20000, 20000, 10000  # appended zero-row index per table
NBANDS = 4
BOUNDS = [0, 1216, 2432, 3648, S]  # short last band -> short tail chain
BSZ = [BOUNDS[i + 1] - BOUNDS[i] for i in range(NBANDS)]
BAND = S // NBANDS  # legacy trash-row fill for _pack helpers

LAST_EXEC_NS = None
LAST_RESULT = None


def _pack16(vals: np.ndarray, n_lanes: int, fill) -> np.ndarray:
    """Pad to n_lanes and pack int16 indices as [128, n_lanes//16]:
    index i lives at [i % 16, i // 16], replicated across the 8 GpSimd
    Q7 cores' 16-partition groups (each core reads its own group)."""
    flat = np.asarray(fill, np.int16) * np.ones(n_lanes, np.int16)
    flat[: len(vals)] = vals.astype(np.int16)
    return np.ascontiguousarray(np.tile(flat.reshape(-1, 16).T, (8, 1)))


def _pack128(vals: np.ndarray, n_lanes: int, fill) -> np.ndarray:
    """Pad to n_lanes and pack int32 as [128, n_lanes//128]: lane i at
    [i % 128, i // 128] (indirect-DMA offset layout)."""
    flat = np.asarray(fill, np.int32) * np.ones(n_lanes, np.int32)
    flat[: len(vals)] = vals.astype(np.int32)
    return np.ascontiguousarray(flat.reshape(-1, 128).T)


def _prep_core(ids_k: np.ndarray):
    out = []
    for lo, hi, zrow in ((0, CUT0, Z0), (CUT0, CUT1, Z1), (CUT1, VOCAB, Z2)):
        m = (ids_k >= lo) & (ids_k < hi)
        pos_all = np.nonzero(m)[0].astype(np.int32)
        loc_all = (ids_k[pos_all].astype(np.int64) - lo).astype(np.int32)
        loc_all[loc_all == PAD] = zrow
        bands = []
        for b in range(NBANDS):
            sel = (pos_all >= BOUNDS[b]) & (pos_all < BOUNDS[b + 1])
            bands.append((loc_all[sel], pos_all[sel] - BOUNDS[b]))
        out.append(bands)
    return out


def _prepare(input_ids: np.ndarray):
    """Returns ((L0, L1, L2), in_maps)."""
    preps = [_prep_core(input_ids[k]) for k in range(N_CORES)]
    L = []
    for c in range(3):
        L.append(
            [
                max(
                    1,
                    -(-max(len(preps[k][c][b][0]) for k in range(N_CORES)) // 128),
                )
                * 128
                for b in range(NBANDS)
            ]
        )

    in_maps = []
    for k in range(N_CORES):
        m = {}
        for c, zrow in ((0, Z0), (1, Z1), (2, Z2)):
            ic, qc = [], []
            for b in range(NBANDS):
                loc, pos = preps[k][c][b]
                ic.append(_pack16(loc, L[c][b], zrow))
                qc.append(_pack128(pos, L[c][b], BSZ[b]))  # pad -> trash row
            m[f"idx{c}"] = np.concatenate(ic, axis=1)
            m[f"pos{c}"] = np.concatenate(qc, axis=1)
        in_maps.append(m)
    return L, in_maps


def _build(nc, L0: int, L1: int, L2: int):
    from concourse import library_config, mybir, tile
    from concourse.bass import IndirectOffsetOnAxis

    f32 = mybir.dt.float32
    bf16 = mybir.dt.bfloat16
    i16 = mybir.dt.int16
    i32 = mybir.dt.int32

    Ls = {0: L0, 1: L1, 2: L2}  # per-band lane counts per cluster
    # prefix sums for idx (cols of 16 lanes) and pos (cols of 128 lanes)
    ioff = {c: [sum(Ls[c][:b]) // 16 for b in range(NBANDS + 1)] for c in Ls}
    poff = {c: [sum(Ls[c][:b]) // 128 for b in range(NBANDS + 1)] for c in Ls}

    emb0p = nc.dram_tensor("emb0p", [Z0 + 1, D], bf16, kind="ExternalInput")
    emb1b = nc.dram_tensor("emb1b", [Z1 + 1, D1], bf16, kind="ExternalInput")
    emb2b = nc.dram_tensor("emb2b", [Z2 + 1, 128], bf16, kind="ExternalInput")
    p1t = nc.dram_tensor("p1t", [D1, D], bf16, kind="ExternalInput")
    p2t = nc.dram_tensor("p2t", [128, D], bf16, kind="ExternalInput")
    idx0 = nc.dram_tensor("idx0", [128, ioff[0][-1]], i16, kind="ExternalInput")
    pos0 = nc.dram_tensor("pos0", [128, poff[0][-1]], i32, kind="ExternalInput")
    idx1 = nc.dram_tensor("idx1", [128, ioff[1][-1]], i16, kind="ExternalInput")
    pos1 = nc.dram_tensor("pos1", [128, poff[1][-1]], i32, kind="ExternalInput")
    idx2 = nc.dram_tensor("idx2", [128, ioff[2][-1]], i16, kind="ExternalInput")
    pos2 = nc.dram_tensor("pos2", [128, poff[2][-1]], i32, kind="ExternalInput")
    outs = [
        nc.dram_tensor(f"out{b}", [BSZ[b] + 1, D], f32, kind="ExternalOutput")
        for b in range(NBANDS)
    ]

    nc.gpsimd.load_library(library_config.mlp)

    with tile.TileContext(nc) as tc:
        with (
            tc.tile_pool(name="const", bufs=1) as cpool,
            tc.tile_pool(name="gA", bufs=3) as gapool,
            tc.tile_pool(name="gB", bufs=2) as gbpool,
            tc.tile_pool(name="o", bufs=3) as opool,
            tc.tile_pool(name="po", bufs=4, space="PSUM") as popool,
        ):
            sb = {}
            for nm, t in (
                ("idx0", idx0),
                ("pos0", pos0),
                ("idx1", idx1),
                ("pos1", pos1),
                ("idx2", idx2),
                ("pos2", pos2),
            ):
                s = cpool.tile(list(t.shape), t.dtype, name=f"{nm}_sb")
                nc.sync.dma_start(out=s[:], in_=t[:])
                sb[nm] = s

            p1b = cpool.tile([128, 2, D], bf16)
            nc.sync.dma_start(out=p1b[:, 0, :], in_=p1t[0:128, :])
            nc.sync.dma_start(out=p1b[:, 1, :], in_=p1t[128:256, :])
            p2b = cpool.tile([128, 1, D], bf16)
            nc.sync.dma_start(out=p2b[:, 0, :], in_=p2t[:])

            for b in range(NBANDS):
                band = outs[b]
                n0b = Ls[0][b] // 128

                # ---- cluster 0: gather bf16 rows, plain indirect scatters ----
                gA = gapool.tile([128, n0b, D], bf16, tag="gA", name=f"gA_{b}")
                nc.gpsimd.dma_gather(
                    gA[:],
                    emb0p[:],
                    sb["idx0"][:, ioff[0][b] : ioff[0][b + 1]],
                    Ls[0][b],
                    Ls[0][b],
                    D,
                )
                for j in range(n0b):
                    nc.gpsimd.indirect_dma_start(
                        out=band[:],
                        out_offset=IndirectOffsetOnAxis(
                            ap=sb["pos0"][:, poff[0][b] + j : poff[0][b] + j + 1],
                            axis=0,
                        ),
                        in_=gA[:, j, :],
                        in_offset=None,
                    )

                # ---- clusters 1/2: transposed gather -> matmul -> scatter ----
                for c, (kch, table, pb, elem) in enumerate(
                    (
                        (2, emb1b, p1b, D1),
                        (1, emb2b, p2b, 128),
                    ),
                    start=1,
                ):
                    lanes = Ls[c][b]
                    n_t = lanes // 128
                    gB = gbpool.tile(
                        [128, kch, lanes], bf16, tag=f"gB{c}", name=f"gB{c}_{b}"
                    )
                    nc.gpsimd.dma_gather(
                        gB[:],
                        table[:],
                        sb[f"idx{c}"][:, ioff[c][b] : ioff[c][b + 1]],
                        lanes,
                        lanes,
                        elem,
                        transpose=True,
                    )
                    oc = opool.tile(
                        [128, n_t, D], f32, tag=f"oc{c}", name=f"oc{c}_{b}"
                    )
                    for t in range(n_t):
                        for nn in range(2):
                            om = popool.tile([128, 512], f32, tag="om", name="om")
                            for kc in range(kch):
                                nc.tensor.matmul(
                                    out=om[:],
                                    lhsT=gB[:, kc, t * 128 : (t + 1) * 128],
                                    rhs=pb[:, kc, nn * 512 : (nn + 1) * 512],
                                    start=(kc == 0),
                                    stop=(kc == kch - 1),
                                )
                            dst = oc[:, t, nn * 512 : (nn + 1) * 512]
                            if nn == 0:
                                nc.scalar.copy(out=dst, in_=om[:])
                            else:
                                nc.vector.tensor_copy(out=dst, in_=om[:])
                    for t in range(n_t):
                        nc.gpsimd.indirect_dma_start(
                            out=band[:],
                            out_offset=IndirectOffsetOnAxis(
                                ap=sb[f"pos{c}"][
                                    :, poff[c][b] + t : poff[c][b] + t + 1
                                ],
                                axis=0,
                            ),
                            in_=oc[:, t, :],
                            in_offset=None,
                        )

    return outs


def kernel(input_ids, emb0, emb1, emb2, proj1, proj2):
    global LAST_EXEC_NS, LAST_RESULT
    import ml_dtypes
    from concourse import bacc
    from concourse.bass_utils import run_bass_kernel_spmd

    bf = ml_dtypes.bfloat16
    input_ids = np.asarray(input_ids)
    assert input_ids.shape == (B, S), input_ids.shape

    emb0p = np.concatenate([emb0, np.zeros((1, D), np.float32)], axis=0).astype(bf)
    emb1b = np.concatenate([emb1, np.zeros((1, D1), np.float32)], axis=0).astype(bf)
    emb2b = np.zeros((Z2 + 1, 128), dtype=bf)
    emb2b[:Z2, :D2] = emb2.astype(bf)
    p1t = np.ascontiguousarray(proj1.T).astype(bf)
    p2t = np.zeros((128, D), dtype=bf)
    p2t[:D2] = np.ascontiguousarray(proj2.T).astype(bf)

    (L0, L1, L2), in_maps = _prepare(input_ids)
    tables = {
        "emb0p": emb0p,
        "emb1b": emb1b,
        "emb2b": emb2b,
        "p1t": p1t,
        "p2t": p2t,
    }
    for m in in_maps:
        m.update(tables)

    nc = bacc.Bacc("TRN2", target_bir_lowering=False, debug=False, num_devices=N_CORES)
    _build(nc, L0, L1, L2)
    nc.compile()

    trace = bool(os.environ.get("EMB_KERNEL_TRACE"))
    res = run_bass_kernel_spmd(nc, in_maps, list(range(N_CORES)), trace=trace)
    LAST_RESULT = res
    LAST_EXEC_NS = res.exec_time_ns

    out = np.stack(
        [
            np.concatenate(
                [
                    np.asarray(res.results[k][f"out{b}"]).reshape(BSZ[b] + 1, D)[: BSZ[b]]
                    for b in range(NBANDS)
                ],
                axis=0,
            )
            for k in range(N_CORES)
        ],
        axis=0,
    )
    return out
